# revision 12
# baseline (speedup 1.0000x reference)
"""Diagonal-matrix multiply (column scale) on 8 Trainium2 NeuronCores.

Computes y = x * weight[None, :]  for x:[8192,4096] f32, weight:[4096] f32.
Data-parallel: rows of x sharded 8 ways (1024 rows/core); weight replicated.

Per-core Bass/Tile kernel: stream [128, SUPER*4096] tiles HBM->SBUF,
multiply by a partition-broadcast copy of weight on the vector engine,
stream back. Memory-bound: ~33.5 MB of HBM traffic per core.
"""

import numpy as np

import concourse.bacc as bacc
import concourse.mybir as mybir
from concourse.tile import TileContext
from concourse.bass_utils import run_bass_kernel_spmd

N_CORES = 8
ROWS = 8192
N = 4096
SHARD_ROWS = ROWS // N_CORES  # 1024
P = 128                       # SBUF partitions
SUPER = 1                     # row-blocks fused per tile -> [128, 1, 4096] (2 MiB)
N_TILES = SHARD_ROWS // (P * SUPER)  # 4 super-tiles per core

_nc_cache = {}


def _build(repeat=1, super_=SUPER, split_rings=True, bufs=None):
    """Build (and Bacc-compile) the per-core kernel.

    repeat > 1 wraps the streaming body in a Tile For_i loop that re-runs
    it `repeat` times (idempotent; for wall-clock timing only).
    split_rings: issue stores on the Activation HWDGE ring (qActDynamicHW)
    so they overlap with loads on the SP ring (qSPDynamicHW).
    """
    key = (repeat, super_, split_rings, bufs)
    if key in _nc_cache:
        return _nc_cache[key]
    n_tiles = SHARD_ROWS // (P * super_)
    if bufs is None:
        bufs = n_tiles
    nc = bacc.Bacc()
    x = nc.dram_tensor("x", [SHARD_ROWS, N], mybir.dt.float32, kind="ExternalInput")
    w = nc.dram_tensor("weight", [N], mybir.dt.float32, kind="ExternalInput")
    y = nc.dram_tensor("y", [SHARD_ROWS, N], mybir.dt.float32, kind="ExternalOutput")

    # partition p of super-tile i holds rows {(super_*i+j)*128 + p : j < super_}
    xv = x.rearrange("(n p) m -> p n m", p=P)  # [128, 8, 4096] view
    yv = y.rearrange("(n p) m -> p n m", p=P)

    store_eng = nc.scalar if split_rings else nc.sync

    with TileContext(nc) as tc:
        with (
            tc.tile_pool(name="const", bufs=1) as cpool,
            tc.tile_pool(name="work", bufs=bufs) as pool,
        ):
            wtile = cpool.tile([P, N], mybir.dt.float32)
            scratch = cpool.tile([P, 1], mybir.dt.float32)
            # replicate weight into every partition (step-0 partition AP)
            nc.gpsimd.dma_start(out=wtile[:, :], in_=w[None, :].to_broadcast([P, N]))
            # tiny DVE read of wtile: advances DVE's observed tick for the
            # weight DMA sem so the muls below don't each need a second
            # sync-wait (DVE TensorTensor supports only one).
            nc.vector.tensor_copy(out=scratch[:, :], in_=wtile[:, :1])
            wb = wtile[:, None, :].to_broadcast([P, super_, N])

            def body():
                for i in range(n_tiles):
                    t = pool.tile([P, super_, N], mybir.dt.float32)
                    nc.sync.dma_start(
                        out=t[:, :, :], in_=xv[:, super_ * i:super_ * (i + 1), :]
                    )
                    nc.vector.tensor_mul(out=t[:, :, :], in0=t[:, :, :], in1=wb)
                    store_eng.dma_start(
                        out=yv[:, super_ * i:super_ * (i + 1), :], in_=t[:, :, :]
                    )

            if repeat == 1:
                body()
            else:
                with tc.For_i(0, repeat, 1):
                    body()
    nc.compile()
    _nc_cache[key] = nc
    return nc


def _shard_inputs(x, weight):
    x = np.ascontiguousarray(np.asarray(x, dtype=np.float32))
    weight = np.ascontiguousarray(np.asarray(weight, dtype=np.float32))
    shards = np.split(x, N_CORES, axis=0)
    return [{"x": s, "weight": weight} for s in shards]


def _run(x, weight, repeat=1, **spmd_kwargs):
    nc = _build(repeat)
    in_maps = _shard_inputs(x, weight)
    res = run_bass_kernel_spmd(nc, in_maps, list(range(N_CORES)), **spmd_kwargs)
    out = np.concatenate([np.asarray(r["y"]) for r in res.results], axis=0)
    return out.astype(np.float32, copy=False), res


def kernel(x, weight):
    out, _ = _run(x, weight)
    return out


# revision 34
# speedup vs baseline: 1.2008x; 1.2008x over previous
"""Diagonal-matrix multiply (column scale) on 8 Trainium2 NeuronCores.

Computes y = x * weight[None, :]  for x:[8192,4096] f32, weight:[4096] f32.
Data-parallel: rows of x sharded 8 ways (1024 rows/core); weight replicated.

Per-core Bass/Tile kernel: stream [128, SUPER*4096] tiles HBM->SBUF,
multiply by a partition-broadcast copy of weight on the vector engine,
stream back. Memory-bound: ~33.5 MB of HBM traffic per core.
"""

import numpy as np

import concourse.bacc as bacc
import concourse.mybir as mybir
from concourse.tile import TileContext
from concourse.bass_utils import run_bass_kernel_spmd

N_CORES = 8
ROWS = 8192
N = 4096
SHARD_ROWS = ROWS // N_CORES  # 1024
P = 128                       # SBUF partitions
SUPER = 1                     # row-blocks fused per tile -> [128, 1, 4096] (2 MiB)
N_TILES = SHARD_ROWS // (P * SUPER)  # 4 super-tiles per core

_nc_cache = {}


def _build(repeat=1, super_=SUPER, split_rings=True, bufs=None, mode="full", group=1,
           store_rings="alt", load_rings="alt"):
    """Build (and Bacc-compile) the per-core kernel.

    repeat > 1 wraps the streaming body in a Tile For_i loop that re-runs
    it `repeat` times (idempotent; for wall-clock timing only).
    split_rings: issue stores on the Activation HWDGE ring (qActDynamicHW)
    so they overlap with loads on the SP ring (qSPDynamicHW).
    mode: 'full' load+mul+store | 'load' loads only | 'copy' load+store (no mul)
    (non-'full' modes produce wrong output; benchmarking only)
    """
    key = (repeat, super_, split_rings, bufs, mode, group, store_rings, load_rings)
    if key in _nc_cache:
        return _nc_cache[key]
    n_tiles = SHARD_ROWS // (P * super_)
    if bufs is None:
        bufs = n_tiles
    nc = bacc.Bacc()
    x = nc.dram_tensor("x", [SHARD_ROWS, N], mybir.dt.float32, kind="ExternalInput")
    w = nc.dram_tensor("weight", [N], mybir.dt.float32, kind="ExternalInput")
    y = nc.dram_tensor("y", [SHARD_ROWS, N], mybir.dt.float32, kind="ExternalOutput")

    # partition p of super-tile i holds rows {(super_*i+j)*128 + p : j < super_}
    xv = x.rearrange("(n p) m -> p n m", p=P)  # [128, 8, 4096] view
    yv = y.rearrange("(n p) m -> p n m", p=P)

    def ring_list(spec):
        return {
            "alt": [nc.sync, nc.scalar],
            "sp": [nc.sync],
            "act": [nc.scalar],
            "gp": [nc.gpsimd],
            "tri": [nc.sync, nc.scalar, nc.gpsimd],
        }[spec]

    store_engs = ring_list(store_rings if split_rings else "sp")
    load_engs = ring_list(load_rings)

    with TileContext(nc) as tc:
        with (
            tc.tile_pool(name="const", bufs=1) as cpool,
            tc.tile_pool(name="work", bufs=bufs if mode != "full_oop" else 4) as pool,
            tc.tile_pool(name="out", bufs=4) as opool,
        ):
            if mode != "store":
                wtile = cpool.tile([P, N], mybir.dt.float32)
                scratch = cpool.tile([P, 1], mybir.dt.float32)
                # replicate weight into every partition (step-0 partition AP)
                nc.gpsimd.dma_start(
                    out=wtile[:, :], in_=w[None, :].to_broadcast([P, N])
                )
                # tiny DVE read of wtile: advances DVE's observed tick for the
                # weight DMA sem so the muls below don't each need a second
                # sync-wait (DVE TensorTensor supports only one).
                nc.vector.tensor_copy(out=scratch[:, :], in_=wtile[:, :1])
                wb = wtile[:, None, :].to_broadcast([P, super_, N])

            def body():
                if mode == "store":
                    # benchmarking only: stores from memset SBUF tiles
                    for i in range(n_tiles):
                        t = pool.tile([P, super_, N], mybir.dt.float32)
                        nc.vector.memset(t[:, :, :], 1.0)
                        eng = store_engs[i % len(store_engs)]
                        eng.dma_start(
                            out=yv[:, super_ * i:super_ * (i + 1), :], in_=t[:, :, :]
                        )
                    return
                if mode in ("dload", "dstore"):
                    # benchmarking only: two same-direction phases with the
                    # same barrier structure as `phased` (33.5 MB one-way).
                    for _ in range(group):
                        for phase in range(2):
                            for i in range(n_tiles):
                                t = pool.tile([P, super_, N], mybir.dt.float32)
                                if mode == "dload":
                                    eng = load_engs[i % len(load_engs)]
                                    eng.dma_start(
                                        out=t[:, :, :],
                                        in_=xv[:, super_ * i:super_ * (i + 1), :],
                                    )
                                    nc.vector.tensor_copy(
                                        out=scratch[:, :], in_=t[:, 0, :1]
                                    )
                                else:
                                    nc.vector.memset(t[:, :, :], 1.0)
                                    eng = store_engs[i % len(store_engs)]
                                    eng.dma_start(
                                        out=yv[:, super_ * i:super_ * (i + 1), :],
                                        in_=t[:, :, :],
                                    )
                            tc.strict_bb_all_engine_barrier()
                    return
                if mode == "phased":
                    # Phase 1: stream all of x in (pipelined with the muls),
                    # phase 2: stream all of y out. Single-direction HBM
                    # traffic per phase avoids read/write turnaround loss.
                    # Trailing barrier keeps the next pass's loads off the
                    # in-flight stores.
                    for _ in range(group):
                        tiles = []
                        for i in range(n_tiles):
                            t = pool.tile([P, super_, N], mybir.dt.float32)
                            eng = load_engs[i % len(load_engs)]
                            eng.dma_start(
                                out=t[:, :, :],
                                in_=xv[:, super_ * i:super_ * (i + 1), :],
                            )
                            nc.vector.tensor_mul(
                                out=t[:, :, :], in0=t[:, :, :], in1=wb
                            )
                            tiles.append(t)
                        tc.strict_bb_all_engine_barrier()
                        for i, t in enumerate(tiles):
                            eng = store_engs[i % len(store_engs)]
                            eng.dma_start(
                                out=yv[:, super_ * i:super_ * (i + 1), :],
                                in_=t[:, :, :],
                            )
                        tc.strict_bb_all_engine_barrier()
                    return
                for i in range(n_tiles):
                    t = pool.tile([P, super_, N], mybir.dt.float32)
                    eng = load_engs[i % len(load_engs)]
                    eng.dma_start(
                        out=t[:, :, :], in_=xv[:, super_ * i:super_ * (i + 1), :]
                    )
                    st = t
                    if mode in ("full", "loadmul"):
                        nc.vector.tensor_mul(out=t[:, :, :], in0=t[:, :, :], in1=wb)
                    elif mode == "full_oop":
                        o = opool.tile([P, super_, N], mybir.dt.float32)
                        nc.vector.tensor_mul(out=o[:, :, :], in0=t[:, :, :], in1=wb)
                        st = o
                    elif mode == "load":
                        # keep the load live with a tiny DVE read
                        nc.vector.tensor_copy(out=scratch[:, :], in_=t[:, 0, :1])
                    if mode in ("full", "full_oop", "copy"):
                        eng = store_engs[i % len(store_engs)]
                        eng.dma_start(
                            out=yv[:, super_ * i:super_ * (i + 1), :], in_=st[:, :, :]
                        )

            if repeat == 1:
                body()
            else:
                with tc.For_i(0, repeat, 1):
                    body()
    nc.compile()
    _nc_cache[key] = nc
    return nc


def _shard_inputs(x, weight):
    x = np.ascontiguousarray(np.asarray(x, dtype=np.float32))
    weight = np.ascontiguousarray(np.asarray(weight, dtype=np.float32))
    shards = np.split(x, N_CORES, axis=0)
    return [{"x": s, "weight": weight} for s in shards]


def _run(x, weight, repeat=1, **spmd_kwargs):
    nc = _build(repeat)
    in_maps = _shard_inputs(x, weight)
    res = run_bass_kernel_spmd(nc, in_maps, list(range(N_CORES)), **spmd_kwargs)
    out = np.concatenate([np.asarray(r["y"]) for r in res.results], axis=0)
    return out.astype(np.float32, copy=False), res


def kernel(x, weight):
    out, _ = _run(x, weight)
    return out
